# revision 1
# baseline (speedup 1.0000x reference)
"""Trainium2 Bass kernel for the bipartite GNN message-passing encoder.

Math (see reference.py):
  A_r = (adj == r), r = 1..5
  An_r = diag(1/sqrt(Nu)) A_r diag(1/sqrt(Nv))   (exact factorization; the
         Csafe guard in the reference only matters where A==0, contributing 0)
  Hu = relu(sum_r An_r @ W_items_r^T)   [NU, M]
  Hv = relu(sum_r An_r^T @ W_users_r^T) [NI, M]
  U  = relu(Hu @ dense_W^T + relu(u_sideFeat @ u_W1^T + u_b1) @ u_W2^T)
  V  = relu(Hv @ dense_W^T + relu(v_sideFeat @ v_W1^T + v_b1) @ v_W2^T)

Sharding: 4 user-groups x 2 item-groups = 8 cores. Core (a, b) holds the
adjacency block adj[a*1000:(a+1)*1000, b*2000:(b+1)*2000] and computes the
partial Hu^T for its 1000 users (partial over items -> AllReduce over the
pair sharing `a`) and the partial Hv^T for its 2000 items (partial over
users -> AllReduce over the quad sharing `b`, split in two pipelined
halves). Degrees (Nu/Nv) are computed on-device with two small
AllReduces; the inner degree scale rides the mask build (dual-op DVE),
the outer degree scale is applied in pass 2. Pass 2 is computed
redundantly inside each reduce group so the SPMD program has no per-core
constants. The msg_W slices are handed to each core pre-transposed
([R, n, M] layout) as part of the host-side sharding.

Engine layout: the MM stream (masks x W^T) is pure back-to-back matmuls
so the PE HAM clock-gate stays at 2.4 GHz; all remaining transposes
(adj^T, side features, small weights) run on the PE during the prefix
window (while the degree AllReduces are in flight) and finish before the
MM stream starts. No DMA-xbar transposes (they hard-hang the device when
concurrent with collectives, and serialize ~1.2us/tile on the issuing
engine). f32->bf16 conversion on ACT; masks on DVE.
"""

import sys

import numpy as np

if "/opt/trn_rl_repo" not in sys.path:
    sys.path.insert(0, "/opt/trn_rl_repo")

import concourse.bacc as bacc  # noqa: E402
import concourse.mybir as mybir  # noqa: E402
import concourse.tile as tile  # noqa: E402
from concourse.masks import make_identity  # noqa: E402

FP = mybir.dt.float32
BF = mybir.dt.bfloat16
I32 = mybir.dt.int32

NU = NI = 4000
R = 5
M = 256
OUT = 75
SIDE = 64
FDIM = 128

GA, GB = 4, 2  # user groups x item groups
BU = NU // GA  # 1000 users per block
BI = NI // GB  # 2000 items per block
NCORES = GA * GB

AF = mybir.ActivationFunctionType
ALU = mybir.AluOpType

PAIR_GROUPS = [[a * GB, a * GB + 1] for a in range(GA)]  # share users (same a)
QUAD_GROUPS = [[b, GB + b, 2 * GB + b, 3 * GB + b] for b in range(GB)]  # same b


def _ptiles(n, p=128):
    return [(s, min(p, n - s)) for s in range(0, n, p)]


UPT = _ptiles(BU)  # 8 tiles over block users
IPT = _ptiles(BI)  # 16 tiles over block items


def build_program():
    from contextlib import ExitStack

    nc = bacc.Bacc("TRN2", target_bir_lowering=False, debug=False, num_devices=NCORES)

    # ---- I/O ----  (wi/wu arrive pre-transposed: [R, n, M])
    adj_blk = nc.dram_tensor("adj_blk", [BU, BI], I32, kind="ExternalInput")
    wi = nc.dram_tensor("wi", [R, BI, M], FP, kind="ExternalInput")
    wu = nc.dram_tensor("wu", [R, BU, M], FP, kind="ExternalInput")
    uf = nc.dram_tensor("uf", [BU, FDIM], FP, kind="ExternalInput")
    vf = nc.dram_tensor("vf", [BI, FDIM], FP, kind="ExternalInput")
    dw = nc.dram_tensor("dw", [OUT, M], FP, kind="ExternalInput")
    uw1 = nc.dram_tensor("uw1", [SIDE, FDIM], FP, kind="ExternalInput")
    ub1 = nc.dram_tensor("ub1", [SIDE, 1], FP, kind="ExternalInput")
    uw2 = nc.dram_tensor("uw2", [OUT, SIDE], FP, kind="ExternalInput")
    vw1 = nc.dram_tensor("vw1", [SIDE, FDIM], FP, kind="ExternalInput")
    vb1 = nc.dram_tensor("vb1", [SIDE, 1], FP, kind="ExternalInput")
    vw2 = nc.dram_tensor("vw2", [OUT, SIDE], FP, kind="ExternalInput")
    u_out = nc.dram_tensor("u_out", [BU, OUT], FP, kind="ExternalOutput")
    v_out = nc.dram_tensor("v_out", [BI, OUT], FP, kind="ExternalOutput")

    with tile.TileContext(nc) as tc, ExitStack() as ctx:
        res = ctx.enter_context(tc.tile_pool(name="res", bufs=1))
        adjp = ctx.enter_context(tc.tile_pool(name="adjp", bufs=1))
        scr = ctx.enter_context(tc.tile_pool(name="scr", bufs=2))
        dram = ctx.enter_context(tc.tile_pool(name="dram", bufs=1, space="DRAM"))
        ps_cs = tc.alloc_tile_pool(name="ps_cs", bufs=4, space="PSUM")
        ps_tr = tc.alloc_tile_pool(name="ps_tr", bufs=2, space="PSUM")

        ones = res.tile([128, 1], BF, tag="ones")
        nc.gpsimd.memset(ones[:], 1.0)
        ident = res.tile([128, 128], BF, tag="ident")
        make_identity(nc, ident[:])

        # =========== Phase 1: adj load/convert, degrees ===========
        adjb = []  # bf16 [128, 2000] resident
        rd_t = []  # row degree [pu, 1] f32 per user ptile
        cs_ps = [
            ps_cs.tile([1, 500], FP, tag="cs", bufs=4, name="cs") for _ in range(4)
        ]
        for t, (s, pu) in enumerate(UPT):
            ab = res.tile([128, 2000], BF, tag=f"adjb{t}", name="ab")
            adjb.append(ab)
            rd = res.tile([128, 1], FP, tag=f"rd{t}", name="rd")
            rd_t.append(rd)
            rdc = []
            for ci, c in enumerate((0, 1000)):
                ai = scr.tile([128, 1000], I32, tag="ai", bufs=5, name="ai")
                nc.sync.dma_start(out=ai[:pu, :], in_=adj_blk[s : s + pu, c : c + 1000])
                nc.scalar.copy(out=ab[:pu, c : c + 1000], in_=ai[:pu, :])
                # nonzero mask (= min(adj,1)) + row-degree partial via accumulate
                nz = scr.tile([128, 1000], BF, tag="nz", bufs=3, name="nz")
                rc = scr.tile([128, 1], FP, tag="rdc", bufs=3, name="rc")
                nc.vector.tensor_scalar(
                    out=nz[:pu, :], in0=ai[:pu, :], scalar1=1.0,
                    scalar2=None, op0=ALU.min,
                )
                nc.vector.tensor_reduce(
                    out=rc[:pu, :], in_=nz[:pu, :], axis=mybir.AxisListType.X,
                    op=ALU.add,
                )
                rdc.append(rc)
                # column-degree partials accumulate in PSUM over user ptiles
                for hi, h in enumerate((0, 500)):
                    nc.tensor.matmul(
                        cs_ps[ci * 2 + hi][:1, :], lhsT=ones[:pu, :1],
                        rhs=nz[:pu, h : h + 500],
                        start=(t == 0), stop=(t == len(UPT) - 1),
                    )
            nc.vector.tensor_tensor(
                out=rd[:pu, :], in0=rdc[0][:pu, :], in1=rdc[1][:pu, :], op=ALU.add
            )

        # degree AllReduces: row (pair) first -- it alone gates the item side
        dram_rd = dram.tile([BU, 1], FP, tag="dram_rd")
        dram_cd = dram.tile([1, BI], FP, tag="dram_cd")
        dram_rd_red = dram.tile([BU, 1], FP, tag="dram_rd_red")
        dram_cd_red = dram.tile([1, BI], FP, tag="dram_cd_red")
        for t, (s, pu) in enumerate(UPT):
            nc.sync.dma_start(out=dram_rd[s : s + pu, :], in_=rd_t[t][:pu, :])
        nc.gpsimd.collective_compute(
            "AllReduce", ALU.add, replica_groups=PAIR_GROUPS,
            ins=[dram_rd.opt()], outs=[dram_rd_red.opt()],
        )
        for q4 in range(4):
            cde = scr.tile([128, 500], FP, tag="ev", bufs=3, name="cde")
            nc.scalar.copy(out=cde[:1, :], in_=cs_ps[q4][:1, :])
            nc.sync.dma_start(
                out=dram_cd[:, q4 * 500 : (q4 + 1) * 500], in_=cde[:1, :]
            )
        nc.gpsimd.collective_compute(
            "AllReduce", ALU.add, replica_groups=QUAD_GROUPS,
            ins=[dram_cd.opt()], outs=[dram_cd_red.opt()],
        )

        def rsqrt_tiles(src_rows, tiles, nm):
            out = []
            for t, (s, p) in enumerate(tiles):
                raw = scr.tile([128, 1], FP, tag="fraw", name="raw")
                nc.sync.dma_start(out=raw[:p, :], in_=src_rows(s, p))
                m1 = scr.tile([128, 1], FP, tag="fm1", name="m1")
                nc.vector.tensor_scalar(
                    out=m1[:p, :], in0=raw[:p, :], scalar1=1.0, scalar2=None,
                    op0=ALU.max,
                )
                sq = scr.tile([128, 1], FP, tag="fsq", name="sq")
                nc.scalar.sqrt(out=sq[:p, :], in_=m1[:p, :])
                fac = res.tile([128, 1], FP, tag=f"{nm}fac{t}", name="fac")
                nc.vector.reciprocal(out=fac[:p, :], in_=sq[:p, :])
                out.append(fac)
            return out

        a_fac = rsqrt_tiles(lambda s, p: dram_rd_red[s : s + p, :], UPT, "a")

        # =========== Phase 3: adj^T via PE transposes (prefix window) ======
        adjT = []  # bf16 [128, 1000] per item ptile
        for t, (s, pi) in enumerate(IPT):
            at = adjp.tile([128, 1000], BF, tag=f"adjT{t}", name="at")
            adjT.append(at)
            pt_ps = ps_tr.tile([128, 1024], BF, tag="trp", name="pt_ps")
            w = 0
            for j, (us, pu) in enumerate(UPT):
                nc.tensor.transpose(
                    pt_ps[:pi, w : w + pu], adjb[j][:pu, s : s + pi], ident[:pu, :pu]
                )
                w += pu
            nc.scalar.copy(out=at[:pi, :], in_=pt_ps[:pi, :BU])

        # =========== Phase 2: W load+convert (pre-transposed on host) ======
        def prep_w(w_dram, tiles, nm):
            outT = [[None for _ in tiles] for _ in range(R)]
            for r in range(R):
                for kt, (s, p) in enumerate(tiles):
                    wf = scr.tile([128, 256], FP, tag="wf", bufs=4, name="wf")
                    nc.scalar.dma_start(out=wf[:p, :], in_=w_dram[r, s : s + p, :])
                    wt = res.tile([128, 256], BF, tag=f"{nm}T{r}_{kt}", name="wt")
                    outT[r][kt] = wt
                    nc.scalar.copy(out=wt[:p, :], in_=wf[:p, :])
            return outT

        wuT = prep_w(wu, UPT, "wu")
        wiT = prep_w(wi, IPT, "wi")

        # release prefix PSUM pools; open MM pool
        ps_tr.release()
        ps_cs.release()
        ps_mm = tc.alloc_tile_pool(name="ps_mm", bufs=4, space="PSUM")

        # DRAM buffers for pass-1 partials
        ICPS = [(0, 1024), (1024, 976)]  # item column splits (ptile-aligned)
        dram_hvT = [
            dram.tile([M, w], FP, tag=f"dram_hvT{i}", name="dhv")
            for i, (c0, w) in enumerate(ICPS)
        ]
        dram_hvT_red = [
            dram.tile([M, w], FP, tag=f"dram_hvT_red{i}", name="dhvr")
            for i, (c0, w) in enumerate(ICPS)
        ]
        dram_huT = dram.tile([M, BU], FP, tag="dram_huT")
        dram_huT_red = dram.tile([M, BU], FP, tag="dram_huT_red")

        # =========== ITEM-side pass 1 ===========
        # HvT[m, i] partial = sum_r sum_u (a_u * mask_r[u,i]) * Wu[r][m,u]
        for icp, (ic0, icw) in enumerate(ICPS):
            chs = [(0, 512), (512, icw - 512)]
            P = [
                [
                    ps_mm.tile([128, 512], FP, tag="p1", bufs=4, name="P")
                    for _ in range(2)
                ]
                for _ in range(2)
            ]
            for r in range(R):
                for kt, (us, pu) in enumerate(UPT):
                    msk = scr.tile([128, 1024], BF, tag="mask", bufs=3, name="msk")
                    nc.vector.tensor_scalar(
                        out=msk[:pu, :icw], in0=adjb[kt][:pu, ic0 : ic0 + icw],
                        scalar1=float(r + 1), scalar2=a_fac[kt][:pu, :],
                        op0=ALU.is_equal, op1=ALU.mult,
                    )
                    first = r == 0 and kt == 0
                    last = r == R - 1 and kt == len(UPT) - 1
                    for mh in range(2):
                        for ic2, (cs0, cw) in enumerate(chs):
                            nc.tensor.matmul(
                                P[ic2][mh][:, :cw],
                                lhsT=wuT[r][kt][:pu, mh * 128 : (mh + 1) * 128],
                                rhs=msk[:pu, cs0 : cs0 + cw],
                                start=first, stop=last,
                            )
            for ic2, (cs0, cw) in enumerate(chs):
                for mh in range(2):
                    ev = scr.tile([128, 512], FP, tag="ev", bufs=3, name="ev")
                    nc.vector.tensor_copy(out=ev[:, :cw], in_=P[ic2][mh][:, :cw])
                    nc.sync.dma_start(
                        out=dram_hvT[icp][
                            mh * 128 : (mh + 1) * 128, cs0 : cs0 + cw
                        ],
                        in_=ev[:, :cw],
                    )
            nc.gpsimd.collective_compute(
                "AllReduce", ALU.add, replica_groups=QUAD_GROUPS,
                ins=[dram_hvT[icp].opt()], outs=[dram_hvT_red[icp].opt()],
            )

        # =========== USER-side pass 1 ===========
        # (b_fac emitted here so its DVE ops don't block the item-side mask
        #  stream in the strict-FIFO DVE queue while the coldeg AR is in
        #  flight)
        b_fac = rsqrt_tiles(lambda s, p: dram_cd_red[:, s : s + p], IPT, "b")
        # HuT[m, u] partial = sum_r sum_i (b_i * maskT_r[i,u]) * Wi[r][m,i]
        P = [
            [ps_mm.tile([128, 500], FP, tag="p1", bufs=4, name="P") for _ in range(2)]
            for _ in range(2)
        ]
        for r in range(R):
            for kt, (isrt, pi) in enumerate(IPT):
                msk = scr.tile([128, 1000], BF, tag="mask", bufs=3, name="msk")
                nc.vector.tensor_scalar(
                    out=msk[:pi, :], in0=adjT[kt][:pi, :],
                    scalar1=float(r + 1), scalar2=b_fac[kt][:pi, :],
                    op0=ALU.is_equal, op1=ALU.mult,
                )
                first = r == 0 and kt == 0
                last = r == R - 1 and kt == len(IPT) - 1
                for mh in range(2):
                    for uc in range(2):
                        nc.tensor.matmul(
                            P[uc][mh][:, :],
                            lhsT=wiT[r][kt][:pi, mh * 128 : (mh + 1) * 128],
                            rhs=msk[:pi, uc * 500 : uc * 500 + 500],
                            start=first, stop=last,
                        )
        for uc in range(2):
            for mh in range(2):
                ev = scr.tile([128, 500], FP, tag="ev", bufs=3, name="ev")
                nc.vector.tensor_copy(out=ev[:, :], in_=P[uc][mh][:, :])
                nc.sync.dma_start(
                    out=dram_huT[mh * 128 : (mh + 1) * 128, uc * 500 : uc * 500 + 500],
                    in_=ev[:, :],
                )
        nc.gpsimd.collective_compute(
            "AllReduce", ALU.add, replica_groups=PAIR_GROUPS,
            ins=[dram_huT.opt()], outs=[dram_huT_red.opt()],
        )

        # release MM PSUM pool, open pass-2 pool
        ps_mm.release()
        ps_p2 = ctx.enter_context(tc.tile_pool(name="ps_p2", bufs=2, space="PSUM"))

        # ===== Pass-2 small-weight + side-feature prep (tail; uses PE) =====
        def load_t_small(w_dram, rows, cols, nm):
            f = scr.tile([128, 128], FP, tag="smf", name="smf")
            nc.sync.dma_start(out=f[:rows, :cols], in_=w_dram[:, :])
            bmat = scr.tile([128, 128], BF, tag="smb", name="smb")
            nc.scalar.copy(out=bmat[:rows, :cols], in_=f[:rows, :cols])
            pt_ps = ps_p2.tile([128, 1024], BF, tag="trp2", name="pt_ps")
            nc.tensor.transpose(
                pt_ps[:cols, :rows], bmat[:rows, :cols], ident[:rows, :rows]
            )
            outt = res.tile([128, max(rows, 8)], BF, tag=f"smT{nm}", name="outt")
            nc.scalar.copy(out=outt[:cols, :rows], in_=pt_ps[:cols, :rows])
            return outt

        dwT = []  # dense_W^T as two [128, OUT] tiles
        for mh in range(2):
            f = scr.tile([128, 128], FP, tag="smf", name="smf")
            nc.sync.dma_start(out=f[:OUT, :128], in_=dw[:, mh * 128 : (mh + 1) * 128])
            bmat = scr.tile([128, 128], BF, tag="smb", name="smb")
            nc.scalar.copy(out=bmat[:OUT, :128], in_=f[:OUT, :128])
            pt_ps = ps_p2.tile([128, 1024], BF, tag="trp2", name="pt_ps")
            nc.tensor.transpose(pt_ps[:128, :OUT], bmat[:OUT, :128], ident[:OUT, :OUT])
            t = res.tile([128, OUT], BF, tag=f"dwT{mh}", name="t")
            nc.scalar.copy(out=t[:, :], in_=pt_ps[:128, :OUT])
            dwT.append(t)

        uw1T = load_t_small(uw1, SIDE, FDIM, "uw1")  # [FDIM, SIDE]
        uw2T = load_t_small(uw2, OUT, SIDE, "uw2")  # [SIDE, OUT]
        vw1T = load_t_small(vw1, SIDE, FDIM, "vw1")
        vw2T = load_t_small(vw2, OUT, SIDE, "vw2")
        ub1_t = res.tile([SIDE, 1], FP, tag="biasu")
        nc.sync.dma_start(out=ub1_t[:, :], in_=ub1[:, :])
        vb1_t = res.tile([SIDE, 1], FP, tag="biasv")
        nc.sync.dma_start(out=vb1_t[:, :], in_=vb1[:, :])

        # side-feature transposes: sfT = bf16(sideFeat)^T [FDIM, n]
        def prep_sfT(side_dram, tiles, n, nm):
            sfT = res.tile([128, n], BF, tag=f"sfT{nm}", name="sfT")
            for g in range(0, len(tiles), 8):
                pt_ps = ps_p2.tile([128, 1024], BF, tag="trp2", name="pt_ps")
                w = 0
                g0 = tiles[g][0]
                for t in range(g, min(g + 8, len(tiles))):
                    s, p = tiles[t]
                    f = scr.tile([128, FDIM], FP, tag="p2f", name="f")
                    nc.sync.dma_start(out=f[:p, :], in_=side_dram[s : s + p, :])
                    bmat = scr.tile([128, FDIM], BF, tag="p2b", name="bmat")
                    nc.scalar.copy(out=bmat[:p, :], in_=f[:p, :])
                    nc.tensor.transpose(
                        pt_ps[:FDIM, w : w + p], bmat[:p, :], ident[:p, :p]
                    )
                    w += p
                nc.scalar.copy(out=sfT[:FDIM, g0 : g0 + w], in_=pt_ps[:FDIM, :w])
            return sfT

        sfT_v = prep_sfT(vf, IPT, BI, "v")
        sfT_u = prep_sfT(uf, UPT, BU, "u")


        def pass2(h_red_parts, sfT, w1T, bias_t, w2T, fac, tiles, n, o_dram, nm):
            # F^T = relu(w1 @ sf^T + b)  [SIDE, n] bf16
            fT = res.tile([SIDE, n], BF, tag=f"fT{nm}", name="fT")
            for c in range(0, n, 500):
                pf = ps_p2.tile([SIDE, 500], FP, tag="pf", name="pf")
                nc.tensor.matmul(
                    pf[:, :], lhsT=w1T[:FDIM, :SIDE], rhs=sfT[:FDIM, c : c + 500],
                    start=True, stop=True,
                )
                nc.scalar.activation(
                    out=fT[:, c : c + 500], in_=pf[:, :], func=AF.Relu,
                    bias=bias_t[:, :],
                )
            # consume each reduced part as it lands
            for dtile, c0, w in h_red_parts:
                hT = []
                for mh in range(2):
                    hf = scr.tile([128, 1024], FP, tag="p2h", name="hf")
                    nc.sync.dma_start(
                        out=hf[:, :w], in_=dtile[mh * 128 : (mh + 1) * 128, :w]
                    )
                    hb = scr.tile([128, 1024], BF, tag="p2hb", bufs=4, name="hb")
                    nc.scalar.activation(out=hb[:, :w], in_=hf[:, :w], func=AF.Relu)
                    hT.append(hb)
                for t, (s, p) in enumerate(tiles):
                    if not (c0 <= s < c0 + w):
                        continue
                    sl = s - c0
                    pa = ps_p2.tile([128, OUT], FP, tag="pa", name="pa")
                    for mh in range(2):
                        nc.tensor.matmul(
                            pa[:p, :], lhsT=hT[mh][:, sl : sl + p],
                            rhs=dwT[mh][:, :OUT],
                            start=(mh == 0), stop=(mh == 1),
                        )
                    sa = scr.tile([128, OUT], FP, tag="p2sa", name="sa")
                    nc.scalar.activation(
                        out=sa[:p, :], in_=pa[:p, :], func=AF.Copy, scale=fac[t][:p, :]
                    )
                    pb = ps_p2.tile([128, OUT], FP, tag="pb", name="pb")
                    nc.tensor.matmul(
                        pb[:p, :], lhsT=fT[:SIDE, s : s + p], rhs=w2T[:SIDE, :OUT],
                        start=True, stop=True,
                    )
                    so = scr.tile([128, OUT], FP, tag="p2so", name="so")
                    nc.vector.tensor_tensor(
                        out=so[:p, :], in0=pb[:p, :], in1=sa[:p, :], op=ALU.add
                    )
                    ro = scr.tile([128, OUT], FP, tag="p2ro", name="ro")
                    nc.scalar.activation(out=ro[:p, :], in_=so[:p, :], func=AF.Relu)
                    nc.sync.dma_start(out=o_dram[s : s + p, :], in_=ro[:p, :])

        pass2(
            [(dram_hvT_red[0], 0, 1024), (dram_hvT_red[1], 1024, 976)],
            sfT_v, vw1T, vb1_t, vw2T, b_fac, IPT, BI, v_out, "v",
        )
        pass2(
            [(dram_huT_red, 0, 1000)],
            sfT_u, uw1T, ub1_t, uw2T, a_fac, UPT, BU, u_out, "u",
        )

    nc.compile()
    return nc


_CACHE = {}


def _get_program():
    if "nc" not in _CACHE:
        _CACHE["nc"] = build_program()
    return _CACHE["nc"]


def make_in_maps(inputs):
    adj = np.asarray(inputs["adj_matrix"], dtype=np.int32)
    u_sf = np.asarray(inputs["u_sideFeat"], dtype=np.float32)
    v_sf = np.asarray(inputs["v_sideFeat"], dtype=np.float32)
    msg_W = np.asarray(inputs["msg_W"], dtype=np.float32)
    dense_W = np.asarray(inputs["dense_W"], dtype=np.float32)
    u_W1 = np.asarray(inputs["u_W1"], dtype=np.float32)
    u_b1 = np.asarray(inputs["u_b1"], dtype=np.float32).reshape(SIDE, 1)
    u_W2 = np.asarray(inputs["u_W2"], dtype=np.float32)
    v_W1 = np.asarray(inputs["v_W1"], dtype=np.float32)
    v_b1 = np.asarray(inputs["v_b1"], dtype=np.float32).reshape(SIDE, 1)
    v_W2 = np.asarray(inputs["v_W2"], dtype=np.float32)

    in_maps = []
    for a in range(GA):
        for b in range(GB):
            in_maps.append(
                {
                    "adj_blk": np.ascontiguousarray(
                        adj[a * BU : (a + 1) * BU, b * BI : (b + 1) * BI]
                    ),
                    # pre-transposed W slices: [R, n, M]
                    "wi": np.ascontiguousarray(
                        msg_W[:, :, NU + b * BI : NU + (b + 1) * BI].transpose(0, 2, 1)
                    ),
                    "wu": np.ascontiguousarray(
                        msg_W[:, :, a * BU : (a + 1) * BU].transpose(0, 2, 1)
                    ),
                    "uf": np.ascontiguousarray(u_sf[a * BU : (a + 1) * BU]),
                    "vf": np.ascontiguousarray(v_sf[b * BI : (b + 1) * BI]),
                    "dw": dense_W,
                    "uw1": u_W1,
                    "ub1": u_b1,
                    "uw2": u_W2,
                    "vw1": v_W1,
                    "vb1": v_b1,
                    "vw2": v_W2,
                }
            )
    return in_maps


def assemble(results):
    U = np.empty((NU, OUT), np.float32)
    V = np.empty((NI, OUT), np.float32)
    for a in range(GA):
        U[a * BU : (a + 1) * BU] = results[a * GB]["u_out"]
    for b in range(GB):
        V[b * BI : (b + 1) * BI] = results[b]["v_out"]
    return (U, V)


def kernel(**inputs):
    from concourse.bass_utils import run_bass_kernel_spmd

    nc = _get_program()
    res = run_bass_kernel_spmd(nc, make_in_maps(inputs), core_ids=list(range(NCORES)))
    return assemble(res.results)



# revision 4
# speedup vs baseline: 1.4110x; 1.4110x over previous
"""Trainium2 Bass kernel for the bipartite GNN message-passing encoder.

Math (see reference.py):
  A_r = (adj == r), r = 1..5
  An_r = diag(a) A_r diag(b),  a_u = rsqrt(max(Nu,1)), b_i = rsqrt(max(Nv,1))
  Hu = relu(sum_r An_r @ W_items_r^T)   [NU, M]
  Hv = relu(sum_r An_r^T @ W_users_r^T) [NI, M]
  U  = relu(Hu @ dense_W^T + relu(u_sideFeat @ u_W1^T + u_b1) @ u_W2^T)
  V  = relu(Hv @ dense_W^T + relu(v_sideFeat @ v_W1^T + v_b1) @ v_W2^T)

Sharding ("collective-free" 8-way): core c owns users [500c, 500c+500) and
items [500c, 500c+500), padded per-core to 512 (padded global index
512c + j, NP = 4096 total).  Each core holds the FULL contraction data for
its rows: adj[:, I_c] u-major (for Hv) and adj[U_c, :]^T i-major (for Hu),
plus the full msg_W both ways (host-packed bf16).  It computes
HvT[m, own-items] / HuT[m, own-users] completely locally, so there are NO
big AllReduces: only two 16KB degree AllReduces, issued at ~17us and
hidden behind weight DMA.  Degrees ride a fused scalar-engine pass
(t = Relu(1 - adj) with accum_out => per-row zero count; deg = 4096 - zc,
the 96 pad slots cancel exactly).  Masks are built on DVE with the
contraction-side degree factor folded in via the dual-op; the output-side
factor is applied in pass 2 per-partition (own-core factors extracted from
the AllReduce result with a data-driven one-hot select, keeping the
program SPMD-uniform).  The matmul stream is 640 back-to-back 512-col
matmuls (measured ~2.25 cols/ns on this HW); LDWEIGHTS overlaps.

Host-side prep (allowed layout work only: slice/pad/transpose/dtype):
everything arrives bf16, pre-transposed, chunk-packed ([128, 32*S] with
element (p, c*S+j) = src[c*128+p, j]) so every big DMA is one contiguous
descriptor.
"""

import sys

import numpy as np

if "/opt/trn_rl_repo" not in sys.path:
    sys.path.insert(0, "/opt/trn_rl_repo")

import concourse.bacc as bacc  # noqa: E402
import concourse.mybir as mybir  # noqa: E402
import concourse.tile as tile  # noqa: E402

FP = mybir.dt.float32
BF = mybir.dt.bfloat16

NU = NI = 4000
R = 5
M = 256
OUT = 75
SIDE = 64
FDIM = 128

NCORES = 8
SO = 500        # owned users/items per core
SP = 512        # padded owned span
NP = 4096       # padded global span
CH = NP // 128  # 32 chunks of 128 along the contraction dim

AF = mybir.ActivationFunctionType
ALU = mybir.AluOpType
WORLD = [list(range(NCORES))]


def build_program():
    from contextlib import ExitStack

    nc = bacc.Bacc("TRN2", target_bir_lowering=False, debug=False, num_devices=NCORES)

    # ---- I/O ----
    # adj_u: chunk-packed [128, CH*SP] from adj_pad[:, I_c]   (u on partitions)
    # adj_i: chunk-packed [128, CH*SP] from adj_pad[U_c, :]^T (i on partitions)
    adj_u = nc.dram_tensor("adj_u", [128, CH * SP], BF, kind="ExternalInput")
    adj_i = nc.dram_tensor("adj_i", [128, CH * SP], BF, kind="ExternalInput")
    # wu/wi: chunk-packed [R, 128, CH*M] from msg_W slices, pre-transposed
    wu = nc.dram_tensor("wu", [R, 128, CH * M], BF, kind="ExternalInput")
    wi = nc.dram_tensor("wi", [R, 128, CH * M], BF, kind="ExternalInput")
    sfu = nc.dram_tensor("sfu", [FDIM, SP], BF, kind="ExternalInput")
    sfv = nc.dram_tensor("sfv", [FDIM, SP], BF, kind="ExternalInput")
    dwt = nc.dram_tensor("dwt", [M, OUT], BF, kind="ExternalInput")
    uw1t = nc.dram_tensor("uw1t", [FDIM, SIDE], BF, kind="ExternalInput")
    ub1 = nc.dram_tensor("ub1", [SIDE, 1], FP, kind="ExternalInput")
    uw2t = nc.dram_tensor("uw2t", [SIDE, OUT], BF, kind="ExternalInput")
    vw1t = nc.dram_tensor("vw1t", [FDIM, SIDE], BF, kind="ExternalInput")
    vb1 = nc.dram_tensor("vb1", [SIDE, 1], FP, kind="ExternalInput")
    vw2t = nc.dram_tensor("vw2t", [SIDE, OUT], BF, kind="ExternalInput")
    # selb: [128, 4*CH] one-hot select blocks; block j column (4c+j) is 1.0
    selb = nc.dram_tensor("selb", [128, 4 * CH], FP, kind="ExternalInput")
    u_out = nc.dram_tensor("u_out", [SP, OUT], FP, kind="ExternalOutput")
    v_out = nc.dram_tensor("v_out", [SP, OUT], FP, kind="ExternalOutput")

    with tile.TileContext(nc) as tc, ExitStack() as ctx:
        res = ctx.enter_context(tc.tile_pool(name="res", bufs=1))
        scr = ctx.enter_context(tc.tile_pool(name="scr", bufs=2))
        wpool = ctx.enter_context(tc.tile_pool(name="wpool", bufs=6))
        dram = ctx.enter_context(tc.tile_pool(name="dram", bufs=1, space="DRAM"))
        ps_chain = ctx.enter_context(tc.tile_pool(name="ps_chain", bufs=4, space="PSUM"))
        ps_small = ctx.enter_context(tc.tile_pool(name="ps_small", bufs=4, space="PSUM"))

        # ---------- bulk DMA issue (tensor-engine queue; PE idle pre-stream) ----
        # transfer order = priority: adj (degree gate) > smalls > weights
        adj_u_sb = []
        adj_i_sb = []
        for k in range(4):
            t = res.tile([128, 8 * SP], BF, tag=f"adju{k}", name="adju")
            nc.gpsimd.dma_start(out=t[:, :], in_=adj_u[:, k * 8 * SP : (k + 1) * 8 * SP])
            adj_u_sb.append(t)
        for k in range(4):
            t = res.tile([128, 8 * SP], BF, tag=f"adji{k}", name="adji")
            nc.gpsimd.dma_start(out=t[:, :], in_=adj_i[:, k * 8 * SP : (k + 1) * 8 * SP])
            adj_i_sb.append(t)

        sfu_sb = res.tile([FDIM, SP], BF, tag="sfu")
        nc.gpsimd.dma_start(out=sfu_sb[:, :], in_=sfu[:, :])
        sfv_sb = res.tile([FDIM, SP], BF, tag="sfv")
        nc.gpsimd.dma_start(out=sfv_sb[:, :], in_=sfv[:, :])
        dwt_sb = []
        for mh in range(2):
            t = res.tile([128, OUT], BF, tag=f"dwt{mh}")
            nc.gpsimd.dma_start(out=t[:, :], in_=dwt[mh * 128 : (mh + 1) * 128, :])
            dwt_sb.append(t)
        uw1t_sb = res.tile([FDIM, SIDE], BF, tag="uw1t")
        nc.gpsimd.dma_start(out=uw1t_sb[:, :], in_=uw1t[:, :])
        uw2t_sb = res.tile([SIDE, OUT], BF, tag="uw2t")
        nc.gpsimd.dma_start(out=uw2t_sb[:, :], in_=uw2t[:, :])
        vw1t_sb = res.tile([FDIM, SIDE], BF, tag="vw1t")
        nc.gpsimd.dma_start(out=vw1t_sb[:, :], in_=vw1t[:, :])
        vw2t_sb = res.tile([SIDE, OUT], BF, tag="vw2t")
        nc.gpsimd.dma_start(out=vw2t_sb[:, :], in_=vw2t[:, :])
        ub1_sb = res.tile([SIDE, 1], FP, tag="ub1")
        nc.gpsimd.dma_start(out=ub1_sb[:, :], in_=ub1[:, :])
        vb1_sb = res.tile([SIDE, 1], FP, tag="vb1")
        nc.gpsimd.dma_start(out=vb1_sb[:, :], in_=vb1[:, :])
        selb_sb = res.tile([128, 4 * CH], FP, tag="selb")
        nc.gpsimd.dma_start(out=selb_sb[:, :], in_=selb[:, :])

        # weights wu[0..4] + wi[0] on the tensor queue (6 fresh wpool bufs);
        # wi[1..4] go on the sync queue later (their WAR waits must not block
        # the PE instruction stream).
        wtiles = []
        for r in range(R):
            t = wpool.tile([128, CH * M], BF, tag="w", name="wt")
            nc.gpsimd.dma_start(out=t[:, :], in_=wu[r, :, :])
            wtiles.append(t)
        witiles = []
        t = wpool.tile([128, CH * M], BF, tag="w", name="wt")
        nc.gpsimd.dma_start(out=t[:, :], in_=wi[0, :, :])
        witiles.append(t)

        # ---------- degree zero-count pass (ACT) + tiny world AllReduces ------
        zcu = res.tile([128, CH], FP, tag="zcu")
        zci = res.tile([128, CH], FP, tag="zci")
        for c in range(CH):
            tscr = scr.tile([128, SP], FP, tag="tscr", bufs=3, name="tscr")
            nc.scalar.activation(
                out=tscr[:, :], in_=adj_u_sb[c // 8][:, (c % 8) * SP : (c % 8 + 1) * SP],
                func=AF.Relu, scale=-1.0, bias=1.0, accum_out=zcu[:, c : c + 1],
            )
        dram_zcu = dram.tile([128, CH], FP, tag="dram_zcu")
        dram_zcu_red = dram.tile([128, CH], FP, tag="dram_zcu_red")
        nc.sync.dma_start(out=dram_zcu[:, :], in_=zcu[:, :])
        nc.gpsimd.collective_compute(
            "AllReduce", ALU.add, replica_groups=WORLD,
            ins=[dram_zcu.opt()], outs=[dram_zcu_red.opt()],
        )
        for c in range(CH):
            tscr = scr.tile([128, SP], FP, tag="tscr", bufs=3, name="tscr")
            nc.scalar.activation(
                out=tscr[:, :], in_=adj_i_sb[c // 8][:, (c % 8) * SP : (c % 8 + 1) * SP],
                func=AF.Relu, scale=-1.0, bias=1.0, accum_out=zci[:, c : c + 1],
            )
        dram_zci = dram.tile([128, CH], FP, tag="dram_zci")
        dram_zci_red = dram.tile([128, CH], FP, tag="dram_zci_red")
        nc.sync.dma_start(out=dram_zci[:, :], in_=zci[:, :])
        nc.gpsimd.collective_compute(
            "AllReduce", ALU.add, replica_groups=WORLD,
            ins=[dram_zci.opt()], outs=[dram_zci_red.opt()],
        )

        # ---------- side-feature pass-2 prep (independent of collectives) -----
        def side_prep(w1t_sb, b1_sb, sf_sb, w2t_sb, nm):
            pf = ps_small.tile([128, SP], FP, tag="sm", name="pf")
            nc.tensor.matmul(
                pf[:SIDE, :], lhsT=w1t_sb[:, :], rhs=sf_sb[:, :], start=True, stop=True
            )
            fT = res.tile([SIDE, SP], BF, tag=f"fT{nm}", name="fT")
            nc.scalar.activation(
                out=fT[:, :], in_=pf[:SIDE, :], func=AF.Relu, bias=b1_sb[:, :]
            )
            fs = []
            for ic in range(4):
                pfs = ps_small.tile([128, SP], FP, tag="sm", name="pfs")
                nc.tensor.matmul(
                    pfs[:, :OUT], lhsT=fT[:, ic * 128 : (ic + 1) * 128],
                    rhs=w2t_sb[:, :], start=True, stop=True,
                )
                t = res.tile([128, OUT], FP, tag=f"fs{nm}{ic}", name="fs")
                nc.vector.tensor_copy(out=t[:, :], in_=pfs[:, :OUT])
                fs.append(t)
            return fs

        fs_u = side_prep(uw1t_sb, ub1_sb, sfu_sb, uw2t_sb, "u")
        fs_v = side_prep(vw1t_sb, vb1_sb, sfv_sb, vw2t_sb, "v")

        # ---------- degree factors ----------
        def fac_all(dram_red, nm):
            back = res.tile([128, CH], FP, tag=f"zb{nm}", name="back")
            nc.sync.dma_start(out=back[:, :], in_=dram_red[:, :])
            d1 = scr.tile([128, CH], FP, tag="d1", name="d1")
            nc.vector.tensor_scalar(
                out=d1[:, :], in0=back[:, :], scalar1=-1.0, scalar2=float(NP),
                op0=ALU.mult, op1=ALU.add,
            )
            d2 = scr.tile([128, CH], FP, tag="d2", name="d2")
            nc.vector.tensor_scalar(
                out=d2[:, :], in0=d1[:, :], scalar1=1.0, scalar2=None, op0=ALU.max
            )
            d3 = scr.tile([128, CH], FP, tag="d3", name="d3")
            nc.scalar.sqrt(out=d3[:, :], in_=d2[:, :])
            fac = res.tile([128, CH], FP, tag=f"fac{nm}", name="fac")
            nc.vector.reciprocal(out=fac[:, :], in_=d3[:, :])
            return fac

        def fac_own(fac, nm):
            own = res.tile([128, 4], FP, tag=f"own{nm}", name="own")
            for j in range(4):
                tmp = scr.tile([128, CH], FP, tag="ot", bufs=2, name="tmp")
                nc.vector.tensor_tensor(
                    out=tmp[:, :], in0=fac[:, :],
                    in1=selb_sb[:, j * CH : (j + 1) * CH], op=ALU.mult,
                )
                nc.vector.tensor_reduce(
                    out=own[:, j : j + 1], in_=tmp[:, :],
                    axis=mybir.AxisListType.X, op=ALU.add,
                )
            return own

        afac = fac_all(dram_zcu_red, "a")   # users: rides Hv masks
        afac_own = fac_own(afac, "a")       # pass-2 u scale

        # ---------- Hv chain (items out; contraction over all users) ---------
        ps_hv = [ps_chain.tile([128, SP], FP, tag="hv", bufs=2, name="hv") for _ in range(2)]
        bfac = None
        bfac_own = None
        for r in range(R):
            for c in range(CH):
                msk = scr.tile([128, SP], BF, tag="msk", bufs=4, name="msk")
                nc.vector.tensor_scalar(
                    out=msk[:, :], in0=adj_u_sb[c // 8][:, (c % 8) * SP : (c % 8 + 1) * SP],
                    scalar1=float(r + 1), scalar2=afac[:, c : c + 1],
                    op0=ALU.is_equal, op1=ALU.mult,
                )
                for mh in range(2):
                    nc.tensor.matmul(
                        ps_hv[mh][:, :],
                        lhsT=wtiles[r][:, c * M + mh * 128 : c * M + (mh + 1) * 128],
                        rhs=msk[:, :],
                        start=(r == 0 and c == 0), stop=(r == R - 1 and c == CH - 1),
                    )
            if r == 2:
                # emit b-side factor chain mid-stream: its AR is long done, so
                # these DVE/ACT ops slot into gaps without stalling the FIFO
                bfac = fac_all(dram_zci_red, "b")
                bfac_own = fac_own(bfac, "b")

        hb_v = []
        for mh in range(2):
            hb = res.tile([128, SP], BF, tag=f"hbv{mh}", name="hbv")
            nc.scalar.activation(out=hb[:, :], in_=ps_hv[mh][:, :], func=AF.Relu)
            hb_v.append(hb)

        # ---------- Hu chain (users out; contraction over all items) ---------
        # wi[1..4] DMAs ride the sync queue: their WAR waits (wpool reuse)
        # stall only sync, never the PE stream.
        for r in range(1, R):
            t = wpool.tile([128, CH * M], BF, tag="w", name="wt")
            nc.sync.dma_start(out=t[:, :], in_=wi[r, :, :])
            witiles.append(t)

        ps_hu = [ps_chain.tile([128, SP], FP, tag="hu", bufs=2, name="hu") for _ in range(2)]
        for r in range(R):
            for c in range(CH):
                msk = scr.tile([128, SP], BF, tag="msk", bufs=4, name="msk")
                nc.vector.tensor_scalar(
                    out=msk[:, :], in0=adj_i_sb[c // 8][:, (c % 8) * SP : (c % 8 + 1) * SP],
                    scalar1=float(r + 1), scalar2=bfac[:, c : c + 1],
                    op0=ALU.is_equal, op1=ALU.mult,
                )
                for mh in range(2):
                    nc.tensor.matmul(
                        ps_hu[mh][:, :],
                        lhsT=witiles[r][:, c * M + mh * 128 : c * M + (mh + 1) * 128],
                        rhs=msk[:, :],
                        start=(r == 0 and c == 0), stop=(r == R - 1 and c == CH - 1),
                    )
        hb_u = []
        for mh in range(2):
            hb = res.tile([128, SP], BF, tag=f"hbu{mh}", name="hbu")
            nc.scalar.activation(out=hb[:, :], in_=ps_hu[mh][:, :], func=AF.Relu)
            hb_u.append(hb)

        # ---------- pass 2 ----------
        def pass2(hb, fac_own_t, fs, o_dram):
            for ic in range(4):
                pa = ps_small.tile([128, SP], FP, tag="sm", name="pa")
                for mh in range(2):
                    nc.tensor.matmul(
                        pa[:, :OUT], lhsT=hb[mh][:, ic * 128 : (ic + 1) * 128],
                        rhs=dwt_sb[mh][:, :], start=(mh == 0), stop=(mh == 1),
                    )
                sa = scr.tile([128, OUT], FP, tag="sa", bufs=3, name="sa")
                nc.scalar.activation(
                    out=sa[:, :], in_=pa[:, :OUT], func=AF.Copy,
                    scale=fac_own_t[:, ic : ic + 1],
                )
                so = scr.tile([128, OUT], FP, tag="so", bufs=3, name="so")
                nc.vector.tensor_tensor(
                    out=so[:, :], in0=sa[:, :], in1=fs[ic][:, :], op=ALU.add
                )
                ro = scr.tile([128, OUT], FP, tag="ro", bufs=3, name="ro")
                nc.scalar.activation(out=ro[:, :], in_=so[:, :], func=AF.Relu)
                nc.sync.dma_start(
                    out=o_dram[ic * 128 : (ic + 1) * 128, :], in_=ro[:, :]
                )

        pass2(hb_v, bfac_own, fs_v, v_out)
        pass2(hb_u, afac_own, fs_u, u_out)

    nc.compile()
    return nc


_CACHE = {}


def _get_program():
    if "nc" not in _CACHE:
        _CACHE["nc"] = build_program()
    return _CACHE["nc"]


def _pack(x):
    """[NP, S] -> [128, CH*S] with element (p, c*S+j) = x[c*128+p, j]."""
    s = x.shape[1]
    return np.ascontiguousarray(
        x.reshape(CH, 128, s).transpose(1, 0, 2).reshape(128, CH * s)
    )


def _pad_groups(x, axis):
    """Pad per-core groups of SO rows/cols to SP along `axis`."""
    x = np.moveaxis(x, axis, 0)
    n = x.shape[0]
    assert n == NCORES * SO
    shp = (NCORES, SO) + x.shape[1:]
    xg = x.reshape(shp)
    pad = [(0, 0)] * xg.ndim
    pad[1] = (0, SP - SO)
    xp = np.pad(xg, pad)
    out = xp.reshape((NCORES * SP,) + x.shape[1:])
    return np.moveaxis(out, 0, axis)


def make_in_maps(inputs):
    import ml_dtypes

    bf16 = ml_dtypes.bfloat16
    adj = np.asarray(inputs["adj_matrix"], dtype=np.int32)
    u_sf = np.asarray(inputs["u_sideFeat"], dtype=np.float32)
    v_sf = np.asarray(inputs["v_sideFeat"], dtype=np.float32)
    msg_W = np.asarray(inputs["msg_W"], dtype=np.float32)
    dense_W = np.asarray(inputs["dense_W"], dtype=np.float32)

    adjp = _pad_groups(_pad_groups(adj.astype(np.float32), 0), 1)  # [NP, NP]
    adjp = adjp.astype(bf16)

    # shared (identical on every core)
    wu_full = _pad_groups(msg_W[:, :, :NU].transpose(0, 2, 1), 1)  # [R, NP, M]
    wi_full = _pad_groups(msg_W[:, :, NU:].transpose(0, 2, 1), 1)
    wu_pack = np.stack([_pack(wu_full[r].astype(bf16)) for r in range(R)])
    wi_pack = np.stack([_pack(wi_full[r].astype(bf16)) for r in range(R)])
    dwt = np.ascontiguousarray(dense_W.T).astype(bf16)
    uw1t = np.ascontiguousarray(np.asarray(inputs["u_W1"], np.float32).T).astype(bf16)
    uw2t = np.ascontiguousarray(np.asarray(inputs["u_W2"], np.float32).T).astype(bf16)
    vw1t = np.ascontiguousarray(np.asarray(inputs["v_W1"], np.float32).T).astype(bf16)
    vw2t = np.ascontiguousarray(np.asarray(inputs["v_W2"], np.float32).T).astype(bf16)
    ub1 = np.asarray(inputs["u_b1"], np.float32).reshape(SIDE, 1)
    vb1 = np.asarray(inputs["v_b1"], np.float32).reshape(SIDE, 1)

    in_maps = []
    for c in range(NCORES):
        sl = slice(c * SP, (c + 1) * SP)
        selb = np.zeros((128, 4 * CH), np.float32)
        for j in range(4):
            selb[:, j * CH + 4 * c + j] = 1.0
        sfu_p = np.zeros((FDIM, SP), np.float32)
        sfu_p[:, :SO] = u_sf[c * SO : (c + 1) * SO].T
        sfv_p = np.zeros((FDIM, SP), np.float32)
        sfv_p[:, :SO] = v_sf[c * SO : (c + 1) * SO].T
        in_maps.append(
            {
                "adj_u": _pack(np.ascontiguousarray(adjp[:, sl])),
                "adj_i": _pack(np.ascontiguousarray(adjp[sl, :].T)),
                "wu": wu_pack,
                "wi": wi_pack,
                "sfu": sfu_p.astype(bf16),
                "sfv": sfv_p.astype(bf16),
                "dwt": dwt,
                "uw1t": uw1t,
                "ub1": ub1,
                "uw2t": uw2t,
                "vw1t": vw1t,
                "vb1": vb1,
                "vw2t": vw2t,
                "selb": selb,
            }
        )
    return in_maps


def assemble(results):
    U = np.empty((NU, OUT), np.float32)
    V = np.empty((NI, OUT), np.float32)
    for c in range(NCORES):
        U[c * SO : (c + 1) * SO] = results[c]["u_out"][:SO]
        V[c * SO : (c + 1) * SO] = results[c]["v_out"][:SO]
    return (U, V)


def kernel(**inputs):
    from concourse.bass_utils import run_bass_kernel_spmd

    nc = _get_program()
    res = run_bass_kernel_spmd(nc, make_in_maps(inputs), core_ids=list(range(NCORES)))
    return assemble(res.results)


# revision 8
# speedup vs baseline: 1.4924x; 1.0577x over previous
"""Trainium2 Bass kernel for the bipartite GNN message-passing encoder.

Math (see reference.py):
  A_r = (adj == r), r = 1..5
  An_r = diag(a) A_r diag(b),  a_u = rsqrt(max(Nu,1)), b_i = rsqrt(max(Nv,1))
  Hu = relu(sum_r An_r @ W_items_r^T)   [NU, M]
  Hv = relu(sum_r An_r^T @ W_users_r^T) [NI, M]
  U  = relu(Hu @ dense_W^T + relu(u_sideFeat @ u_W1^T + u_b1) @ u_W2^T)
  V  = relu(Hv @ dense_W^T + relu(v_sideFeat @ v_W1^T + v_b1) @ v_W2^T)

Sharding ("collective-free" 8-way): core c owns users [500c, 500c+500) and
items [500c, 500c+500), padded per-core to 512 (padded global index
512c + j, NP = 4096 total).  Each core holds the FULL contraction data for
its rows: adj[:, I_c] u-major (for Hv) and adj[U_c, :]^T i-major (for Hu),
plus the full msg_W both ways (host-packed bf16).  It computes
HvT[m, own-items] / HuT[m, own-users] completely locally, so there are NO
big AllReduces: only two 16KB degree AllReduces, issued at ~17us and
hidden behind weight DMA.  Degrees ride a fused scalar-engine pass
(t = Relu(1 - adj) with accum_out => per-row zero count; deg = 4096 - zc,
the 96 pad slots cancel exactly).  Masks are built on DVE with the
contraction-side degree factor folded in via the dual-op; the output-side
factor is applied in pass 2 per-partition (own-core factors extracted from
the AllReduce result with a data-driven one-hot select, keeping the
program SPMD-uniform).  The matmul stream is 640 back-to-back 512-col
matmuls (measured ~2.25 cols/ns on this HW); LDWEIGHTS overlaps.

Host-side prep (allowed layout work only: slice/pad/transpose/dtype):
everything arrives bf16, pre-transposed, chunk-packed ([128, 32*S] with
element (p, c*S+j) = src[c*128+p, j]) so every big DMA is one contiguous
descriptor.
"""

import sys

import numpy as np

if "/opt/trn_rl_repo" not in sys.path:
    sys.path.insert(0, "/opt/trn_rl_repo")

import concourse.bacc as bacc  # noqa: E402
import concourse.mybir as mybir  # noqa: E402
import concourse.tile as tile  # noqa: E402

FP = mybir.dt.float32
BF = mybir.dt.bfloat16

NU = NI = 4000
R = 5
M = 256
OUT = 75
SIDE = 64
FDIM = 128

NCORES = 8
SO = 500        # owned users/items per core
SP = 512        # padded owned span
NP = 4096       # padded global span
CH = NP // 128  # 32 chunks of 128 along the contraction dim

AF = mybir.ActivationFunctionType
ALU = mybir.AluOpType
WORLD = [list(range(NCORES))]


def build_program():
    from contextlib import ExitStack

    nc = bacc.Bacc("TRN2", target_bir_lowering=False, debug=False, num_devices=NCORES)

    # ---- I/O ----
    # adj_u: chunk-packed [128, CH*SP] from adj_pad[:, I_c]   (u on partitions)
    # adj_i: chunk-packed [128, CH*SP] from adj_pad[U_c, :]^T (i on partitions)
    adj_u = nc.dram_tensor("adj_u", [128, CH * SP], BF, kind="ExternalInput")
    adj_i = nc.dram_tensor("adj_i", [128, CH * SP], BF, kind="ExternalInput")
    # wu/wi: chunk-packed [R, 128, CH*M] from msg_W slices, pre-transposed
    wu = nc.dram_tensor("wu", [R, 128, CH * M], BF, kind="ExternalInput")
    wi = nc.dram_tensor("wi", [R, 128, CH * M], BF, kind="ExternalInput")
    sfu = nc.dram_tensor("sfu", [FDIM, SP], BF, kind="ExternalInput")
    sfv = nc.dram_tensor("sfv", [FDIM, SP], BF, kind="ExternalInput")
    dwt = nc.dram_tensor("dwt", [M, OUT], BF, kind="ExternalInput")
    uw1t = nc.dram_tensor("uw1t", [FDIM, SIDE], BF, kind="ExternalInput")
    ub1 = nc.dram_tensor("ub1", [SIDE, 1], FP, kind="ExternalInput")
    uw2t = nc.dram_tensor("uw2t", [SIDE, OUT], BF, kind="ExternalInput")
    vw1t = nc.dram_tensor("vw1t", [FDIM, SIDE], BF, kind="ExternalInput")
    vb1 = nc.dram_tensor("vb1", [SIDE, 1], FP, kind="ExternalInput")
    vw2t = nc.dram_tensor("vw2t", [SIDE, OUT], BF, kind="ExternalInput")
    # selb: [128, 4*CH] one-hot select blocks; block j column (4c+j) is 1.0
    selb = nc.dram_tensor("selb", [128, 4 * CH], FP, kind="ExternalInput")
    u_out = nc.dram_tensor("u_out", [SO, OUT], FP, kind="ExternalOutput")
    v_out = nc.dram_tensor("v_out", [SO, OUT], FP, kind="ExternalOutput")

    with tile.TileContext(nc) as tc, ExitStack() as ctx:
        res = ctx.enter_context(tc.tile_pool(name="res", bufs=1))
        scr = ctx.enter_context(tc.tile_pool(name="scr", bufs=2))
        wpool = ctx.enter_context(tc.tile_pool(name="wpool", bufs=6))
        dram = ctx.enter_context(tc.tile_pool(name="dram", bufs=1, space="DRAM"))
        ps_chain = ctx.enter_context(tc.tile_pool(name="ps_chain", bufs=4, space="PSUM"))
        ps_small = ctx.enter_context(tc.tile_pool(name="ps_small", bufs=4, space="PSUM"))

        # ---------- bulk DMA issue (tensor-engine queue; PE idle pre-stream) ----
        # transfer order = priority: adj (degree gate) > smalls > weights
        adj_u_sb = []
        adj_i_sb = []
        for k in range(4):
            t = res.tile([128, 8 * SP], BF, tag=f"adju{k}", name="adju")
            nc.gpsimd.dma_start(out=t[:, :], in_=adj_u[:, k * 8 * SP : (k + 1) * 8 * SP])
            adj_u_sb.append(t)
        for k in range(4):
            t = res.tile([128, 8 * SP], BF, tag=f"adji{k}", name="adji")
            nc.gpsimd.dma_start(out=t[:, :], in_=adj_i[:, k * 8 * SP : (k + 1) * 8 * SP])
            adj_i_sb.append(t)

        sfu_sb = res.tile([FDIM, SP], BF, tag="sfu")
        nc.gpsimd.dma_start(out=sfu_sb[:, :], in_=sfu[:, :])
        sfv_sb = res.tile([FDIM, SP], BF, tag="sfv")
        nc.gpsimd.dma_start(out=sfv_sb[:, :], in_=sfv[:, :])
        dwt_sb = []
        for mh in range(2):
            t = res.tile([128, OUT], BF, tag=f"dwt{mh}")
            nc.gpsimd.dma_start(out=t[:, :], in_=dwt[mh * 128 : (mh + 1) * 128, :])
            dwt_sb.append(t)
        uw1t_sb = res.tile([FDIM, SIDE], BF, tag="uw1t")
        nc.gpsimd.dma_start(out=uw1t_sb[:, :], in_=uw1t[:, :])
        uw2t_sb = res.tile([SIDE, OUT], BF, tag="uw2t")
        nc.gpsimd.dma_start(out=uw2t_sb[:, :], in_=uw2t[:, :])
        vw1t_sb = res.tile([FDIM, SIDE], BF, tag="vw1t")
        nc.gpsimd.dma_start(out=vw1t_sb[:, :], in_=vw1t[:, :])
        vw2t_sb = res.tile([SIDE, OUT], BF, tag="vw2t")
        nc.gpsimd.dma_start(out=vw2t_sb[:, :], in_=vw2t[:, :])
        ub1_sb = res.tile([SIDE, 1], FP, tag="ub1")
        nc.gpsimd.dma_start(out=ub1_sb[:, :], in_=ub1[:, :])
        vb1_sb = res.tile([SIDE, 1], FP, tag="vb1")
        nc.gpsimd.dma_start(out=vb1_sb[:, :], in_=vb1[:, :])
        selb_sb = res.tile([128, 4 * CH], FP, tag="selb")
        nc.gpsimd.dma_start(out=selb_sb[:, :], in_=selb[:, :])

        # weights wu[0..4] + wi[0] on the tensor queue (6 fresh wpool bufs);
        # wi[1..4] go on the sync queue later (their WAR waits must not block
        # the PE instruction stream).
        wtiles = []
        for r in range(R):
            t = wpool.tile([128, CH * M], BF, tag="w", name="wt")
            nc.gpsimd.dma_start(out=t[:, :], in_=wu[r, :, :])
            wtiles.append(t)
        witiles = []
        t = wpool.tile([128, CH * M], BF, tag="w", name="wt")
        nc.gpsimd.dma_start(out=t[:, :], in_=wi[0, :, :])
        witiles.append(t)

        # ---------- degree zero-count pass + tiny world AllReduces ------------
        # Split across DVE (is_equal 0) and ACT (Relu(1-x)); both with a
        # free-axis accum_out giving per-row zero counts over the own span.
        # The LAST-started core's zcu path gates the AllReduce (launch skew),
        # so this latency is on the critical path.
        def zc_pass(adj_sb, zc):
            for c in range(CH):
                sl = adj_sb[c // 8][:, (c % 8) * SP : (c % 8) * SP + SO]
                if c % 2 == 0:
                    tscr = scr.tile([128, SO], FP, tag="tscrv", bufs=3, name="tscr")
                    nc.vector.tensor_scalar(
                        out=tscr[:, :], in0=sl, scalar1=0.0, scalar2=None,
                        op0=ALU.is_equal, op1=ALU.add, accum_out=zc[:, c : c + 1],
                    )
                else:
                    tscr = scr.tile([128, SO], FP, tag="tscrs", bufs=3, name="tscr")
                    nc.scalar.activation(
                        out=tscr[:, :], in_=sl,
                        func=AF.Relu, scale=-1.0, bias=1.0,
                        accum_out=zc[:, c : c + 1],
                    )

        zcu = res.tile([128, CH], FP, tag="zcu")
        zci = res.tile([128, CH], FP, tag="zci")
        zc_pass(adj_u_sb, zcu)
        dram_zcu = dram.tile([128, CH], FP, tag="dram_zcu")
        dram_zcu_red = dram.tile([128, CH], FP, tag="dram_zcu_red")
        nc.sync.dma_start(out=dram_zcu[:, :], in_=zcu[:, :])
        nc.gpsimd.collective_compute(
            "AllReduce", ALU.add, replica_groups=WORLD,
            ins=[dram_zcu.opt()], outs=[dram_zcu_red.opt()],
        )
        zc_pass(adj_i_sb, zci)
        dram_zci = dram.tile([128, CH], FP, tag="dram_zci")
        dram_zci_red = dram.tile([128, CH], FP, tag="dram_zci_red")
        nc.sync.dma_start(out=dram_zci[:, :], in_=zci[:, :])
        nc.gpsimd.collective_compute(
            "AllReduce", ALU.add, replica_groups=WORLD,
            ins=[dram_zci.opt()], outs=[dram_zci_red.opt()],
        )

        # ---------- side-feature pass-2 prep (independent of collectives) -----
        def side_prep(w1t_sb, b1_sb, sf_sb, w2t_sb, nm):
            pf = ps_small.tile([128, SP], FP, tag="sm", name="pf")
            nc.tensor.matmul(
                pf[:SIDE, :SO], lhsT=w1t_sb[:, :], rhs=sf_sb[:, :SO],
                start=True, stop=True,
            )
            fT = res.tile([SIDE, SO], BF, tag=f"fT{nm}", name="fT")
            nc.scalar.activation(
                out=fT[:, :], in_=pf[:SIDE, :SO], func=AF.Relu, bias=b1_sb[:, :]
            )
            fs = []
            for ic in range(4):
                w = min(128, SO - ic * 128)
                pfs = ps_small.tile([128, SP], FP, tag="sm", name="pfs")
                nc.tensor.matmul(
                    pfs[:w, :OUT], lhsT=fT[:, ic * 128 : ic * 128 + w],
                    rhs=w2t_sb[:, :], start=True, stop=True,
                )
                t = res.tile([128, OUT], FP, tag=f"fs{nm}{ic}", name="fs")
                nc.vector.tensor_copy(out=t[:w, :], in_=pfs[:w, :OUT])
                fs.append(t)
            return fs

        fs_u = side_prep(uw1t_sb, ub1_sb, sfu_sb, uw2t_sb, "u")
        fs_v = side_prep(vw1t_sb, vb1_sb, sfv_sb, vw2t_sb, "v")

        # ---------- degree factors ----------
        def fac_all(dram_red, nm):
            back = res.tile([128, CH], FP, tag=f"zb{nm}", name="back")
            nc.sync.dma_start(out=back[:, :], in_=dram_red[:, :])
            d1 = scr.tile([128, CH], FP, tag="d1", name="d1")
            nc.vector.tensor_scalar(
                out=d1[:, :], in0=back[:, :], scalar1=-1.0, scalar2=float(NU),
                op0=ALU.mult, op1=ALU.add,
            )
            d2 = scr.tile([128, CH], FP, tag="d2", name="d2")
            nc.vector.tensor_scalar(
                out=d2[:, :], in0=d1[:, :], scalar1=1.0, scalar2=None, op0=ALU.max
            )
            d3 = scr.tile([128, CH], FP, tag="d3", name="d3")
            nc.scalar.sqrt(out=d3[:, :], in_=d2[:, :])
            fac = res.tile([128, CH], FP, tag=f"fac{nm}", name="fac")
            nc.vector.reciprocal(out=fac[:, :], in_=d3[:, :])
            return fac

        def fac_own(fac, nm):
            own = res.tile([128, 4], FP, tag=f"own{nm}", name="own")
            for j in range(4):
                tmp = scr.tile([128, CH], FP, tag="ot", bufs=2, name="tmp")
                nc.vector.tensor_tensor(
                    out=tmp[:, :], in0=fac[:, :],
                    in1=selb_sb[:, j * CH : (j + 1) * CH], op=ALU.mult,
                )
                nc.vector.tensor_reduce(
                    out=own[:, j : j + 1], in_=tmp[:, :],
                    axis=mybir.AxisListType.X, op=ALU.add,
                )
            return own

        afac = fac_all(dram_zcu_red, "a")   # users: rides Hv masks
        afac_own = fac_own(afac, "a")       # pass-2 u scale

        # ---------- Hv chain (items out; contraction over all users) ---------
        ps_hv = [ps_chain.tile([128, SO], FP, tag="hv", bufs=2, name="hv") for _ in range(2)]
        bfac = None
        bfac_own = None
        for r in range(R):
            for c in range(CH):
                msk = scr.tile([128, SO], BF, tag="msk", bufs=4, name="msk")
                nc.vector.tensor_scalar(
                    out=msk[:, :], in0=adj_u_sb[c // 8][:, (c % 8) * SP : (c % 8) * SP + SO],
                    scalar1=float(r + 1), scalar2=afac[:, c : c + 1],
                    op0=ALU.is_equal, op1=ALU.mult,
                )
                for mh in range(2):
                    nc.tensor.matmul(
                        ps_hv[mh][:, :],
                        lhsT=wtiles[r][:, c * M + mh * 128 : c * M + (mh + 1) * 128],
                        rhs=msk[:, :],
                        start=(r == 0 and c == 0), stop=(r == R - 1 and c == CH - 1),
                    )
            if r == 2:
                # emit b-side factor chain mid-stream: its AR is long done, so
                # these DVE/ACT ops slot into gaps without stalling the FIFO
                bfac = fac_all(dram_zci_red, "b")
                bfac_own = fac_own(bfac, "b")

        hb_v = []
        for mh in range(2):
            hb = res.tile([128, SO], BF, tag=f"hbv{mh}", name="hbv")
            nc.scalar.activation(out=hb[:, :], in_=ps_hv[mh][:, :], func=AF.Relu)
            hb_v.append(hb)

        # wi[1..4] DMAs ride the sync queue: their WAR waits (wpool reuse)
        # stall only sync, never the PE stream.
        for r in range(1, R):
            t = wpool.tile([128, CH * M], BF, tag="w", name="wt")
            nc.sync.dma_start(out=t[:, :], in_=wi[r, :, :])
            witiles.append(t)

        # ---------- pass 2 (v emitted between the chains: its latency chain
        # hides under the Hu stream; the adds run on the idle gpsimd engine
        # so the DVE mask FIFO is never blocked) ----------
        def pass2(hb, fac_own_t, fs, o_dram):
            for ic in range(4):
                w = min(128, SO - ic * 128)
                pa = ps_small.tile([128, SP], FP, tag="sm", name="pa")
                for mh in range(2):
                    nc.tensor.matmul(
                        pa[:w, :OUT], lhsT=hb[mh][:, ic * 128 : ic * 128 + w],
                        rhs=dwt_sb[mh][:, :], start=(mh == 0), stop=(mh == 1),
                    )
                sa = scr.tile([128, OUT], FP, tag="sa", bufs=3, name="sa")
                nc.scalar.activation(
                    out=sa[:w, :], in_=pa[:w, :OUT], func=AF.Copy,
                    scale=fac_own_t[:w, ic : ic + 1],
                )
                so = scr.tile([128, OUT], FP, tag="so", bufs=3, name="so")
                nc.gpsimd.tensor_tensor(
                    out=so[:w, :], in0=sa[:w, :], in1=fs[ic][:w, :], op=ALU.add
                )
                ro = scr.tile([128, OUT], FP, tag="ro", bufs=3, name="ro")
                nc.scalar.activation(out=ro[:w, :], in_=so[:w, :], func=AF.Relu)
                nc.sync.dma_start(
                    out=o_dram[ic * 128 : ic * 128 + w, :], in_=ro[:w, :]
                )

        pass2(hb_v, bfac_own, fs_v, v_out)

        # ---------- Hu chain (users out; contraction over all items) ---------
        ps_hu = [ps_chain.tile([128, SO], FP, tag="hu", bufs=2, name="hu") for _ in range(2)]
        for r in range(R):
            for c in range(CH):
                msk = scr.tile([128, SO], BF, tag="msk", bufs=4, name="msk")
                nc.vector.tensor_scalar(
                    out=msk[:, :], in0=adj_i_sb[c // 8][:, (c % 8) * SP : (c % 8) * SP + SO],
                    scalar1=float(r + 1), scalar2=bfac[:, c : c + 1],
                    op0=ALU.is_equal, op1=ALU.mult,
                )
                for mh in range(2):
                    nc.tensor.matmul(
                        ps_hu[mh][:, :],
                        lhsT=witiles[r][:, c * M + mh * 128 : c * M + (mh + 1) * 128],
                        rhs=msk[:, :],
                        start=(r == 0 and c == 0), stop=(r == R - 1 and c == CH - 1),
                    )
        hb_u = []
        for mh in range(2):
            hb = res.tile([128, SO], BF, tag=f"hbu{mh}", name="hbu")
            nc.scalar.activation(out=hb[:, :], in_=ps_hu[mh][:, :], func=AF.Relu)
            hb_u.append(hb)

        pass2(hb_u, afac_own, fs_u, u_out)

    nc.compile()
    return nc


_CACHE = {}


def _get_program():
    if "nc" not in _CACHE:
        _CACHE["nc"] = build_program()
    return _CACHE["nc"]


def _pack(x):
    """[NP, S] -> [128, CH*S] with element (p, c*S+j) = x[c*128+p, j]."""
    s = x.shape[1]
    return np.ascontiguousarray(
        x.reshape(CH, 128, s).transpose(1, 0, 2).reshape(128, CH * s)
    )


def _pad_groups(x, axis):
    """Pad per-core groups of SO rows/cols to SP along `axis`."""
    x = np.moveaxis(x, axis, 0)
    n = x.shape[0]
    assert n == NCORES * SO
    shp = (NCORES, SO) + x.shape[1:]
    xg = x.reshape(shp)
    pad = [(0, 0)] * xg.ndim
    pad[1] = (0, SP - SO)
    xp = np.pad(xg, pad)
    out = xp.reshape((NCORES * SP,) + x.shape[1:])
    return np.moveaxis(out, 0, axis)


def make_in_maps(inputs):
    import ml_dtypes

    bf16 = ml_dtypes.bfloat16
    adj = np.asarray(inputs["adj_matrix"], dtype=np.int32)
    u_sf = np.asarray(inputs["u_sideFeat"], dtype=np.float32)
    v_sf = np.asarray(inputs["v_sideFeat"], dtype=np.float32)
    msg_W = np.asarray(inputs["msg_W"], dtype=np.float32)
    dense_W = np.asarray(inputs["dense_W"], dtype=np.float32)

    adjp = _pad_groups(_pad_groups(adj.astype(np.float32), 0), 1)  # [NP, NP]
    adjp = adjp.astype(bf16)

    # shared (identical on every core)
    wu_full = _pad_groups(msg_W[:, :, :NU].transpose(0, 2, 1), 1)  # [R, NP, M]
    wi_full = _pad_groups(msg_W[:, :, NU:].transpose(0, 2, 1), 1)
    wu_pack = np.stack([_pack(wu_full[r].astype(bf16)) for r in range(R)])
    wi_pack = np.stack([_pack(wi_full[r].astype(bf16)) for r in range(R)])
    dwt = np.ascontiguousarray(dense_W.T).astype(bf16)
    uw1t = np.ascontiguousarray(np.asarray(inputs["u_W1"], np.float32).T).astype(bf16)
    uw2t = np.ascontiguousarray(np.asarray(inputs["u_W2"], np.float32).T).astype(bf16)
    vw1t = np.ascontiguousarray(np.asarray(inputs["v_W1"], np.float32).T).astype(bf16)
    vw2t = np.ascontiguousarray(np.asarray(inputs["v_W2"], np.float32).T).astype(bf16)
    ub1 = np.asarray(inputs["u_b1"], np.float32).reshape(SIDE, 1)
    vb1 = np.asarray(inputs["v_b1"], np.float32).reshape(SIDE, 1)

    in_maps = []
    for c in range(NCORES):
        sl = slice(c * SP, (c + 1) * SP)
        selb = np.zeros((128, 4 * CH), np.float32)
        for j in range(4):
            selb[:, j * CH + 4 * c + j] = 1.0
        sfu_p = np.zeros((FDIM, SP), np.float32)
        sfu_p[:, :SO] = u_sf[c * SO : (c + 1) * SO].T
        sfv_p = np.zeros((FDIM, SP), np.float32)
        sfv_p[:, :SO] = v_sf[c * SO : (c + 1) * SO].T
        in_maps.append(
            {
                "adj_u": _pack(np.ascontiguousarray(adjp[:, sl])),
                "adj_i": _pack(np.ascontiguousarray(adjp[sl, :].T)),
                "wu": wu_pack,
                "wi": wi_pack,
                "sfu": sfu_p.astype(bf16),
                "sfv": sfv_p.astype(bf16),
                "dwt": dwt,
                "uw1t": uw1t,
                "ub1": ub1,
                "uw2t": uw2t,
                "vw1t": vw1t,
                "vb1": vb1,
                "vw2t": vw2t,
                "selb": selb,
            }
        )
    return in_maps


def assemble(results):
    U = np.empty((NU, OUT), np.float32)
    V = np.empty((NI, OUT), np.float32)
    for c in range(NCORES):
        U[c * SO : (c + 1) * SO] = results[c]["u_out"][:SO]
        V[c * SO : (c + 1) * SO] = results[c]["v_out"][:SO]
    return (U, V)


def kernel(**inputs):
    from concourse.bass_utils import run_bass_kernel_spmd

    nc = _get_program()
    res = run_bass_kernel_spmd(nc, make_in_maps(inputs), core_ids=list(range(NCORES)))
    return assemble(res.results)
